# revision 9
# baseline (speedup 1.0000x reference)
"""GraphTransformer (2-layer TransformerConv + MLP) on 8 Trainium2 cores.

Sharding: nodes split 2500/core (padded to 2560 = 20 blocks of 128 dsts).
Edges partitioned by destination core and sorted by destination. Per layer:
each core computes its K/V shard, an AllGather builds the full [20480,1024]
bf16 KV table in every core's DRAM, and dma_gather fetches per-edge KV rows
by (remapped) source index. Attention is computed per 128-dst node block:
one-hot dst matrices are built on-device (is_equal against an iota tile) and
used as PE matmuls for both the q-gather and the segment softmax
scatter-sum (exp without max-subtraction; exact softmax invariance).
The MLP runs in transposed (feature-major) layout so chained matmuls need no
activation transposes; only y (attention output) is transposed once per block.

SBUF multi-slab tensors (>128 rows) are packed as [128, nslab*width] with
slab s at free offset s*width.
"""
import hashlib
import numpy as np
import ml_dtypes

import concourse.bass as bass
import concourse.tile as tile
from concourse import bacc, mybir
from concourse.bass_utils import run_bass_kernel_spmd

F32 = mybir.dt.float32
BF16 = mybir.dt.bfloat16
I16 = mybir.dt.int16
NPBF16 = ml_dtypes.bfloat16

AF = mybir.ActivationFunctionType
ALU = mybir.AluOpType


def full_cfg():
    return dict(N=20000, E=160000, IN_C=256, HID=512, HEADS=8, L=2, NCORES=8)


def _derived(cfg):
    d = dict(cfg)
    d["NPC"] = cfg["N"] // cfg["NCORES"]          # nodes per core (exact)
    assert d["NPC"] * cfg["NCORES"] == cfg["N"]
    d["NB"] = -(-d["NPC"] // 128)                  # node blocks per core
    d["NPCP"] = d["NB"] * 128                      # padded nodes per core
    d["DH"] = cfg["HID"] // cfg["HEADS"]
    return d


# ---------------------------------------------------------------- host prep

def host_prep(edge_index, cfg):
    """Partition+sort edges by destination; build per-core packed index
    tensors and the (core-independent) per-block chunk structure."""
    d = _derived(cfg)
    NC, NPC, NB = cfg["NCORES"], d["NPC"], d["NB"]
    src = np.asarray(edge_index[0]).astype(np.int64)
    dst = np.asarray(edge_index[1]).astype(np.int64)

    per_core_raw = []
    for c in range(NC):
        sel = (dst // NPC) == c
        s_c = src[sel]
        d_c = dst[sel] - c * NPC
        e_c = np.nonzero(sel)[0]                    # original edge ids
        order = np.argsort(d_c, kind="stable")
        per_core_raw.append((s_c[order], d_c[order], e_c[order]))

    # per-(core, block) real edge counts -> shared chunk structure
    counts = np.zeros((NC, NB), dtype=np.int64)
    for c in range(NC):
        _, d_c, _ = per_core_raw[c]
        b = d_c // 128
        for bb, cnt in zip(*np.unique(b, return_counts=True)):
            counts[c, bb] = cnt
    cpb = np.maximum(1, -(-counts.max(axis=0) // 128))  # chunks per block
    block_edges = cpb * 128
    tot_e = int(block_edges.sum())                      # padded edges per core
    block_off = np.concatenate([[0], np.cumsum(block_edges)])[:-1]

    # gather groups: up to 4 chunks (512 edges) per dma_gather call
    groups = []     # list of (block, flat_off, n_idx)
    for b in range(NB):
        off = int(block_off[b])
        rem = int(block_edges[b])
        while rem > 0:
            n = min(512, rem)
            groups.append((b, off, n))
            off += n
            rem -= n

    prep_cores = []
    for c in range(NC):
        s_c, d_c, e_c = per_core_raw[c]
        ag_row = np.zeros(tot_e, dtype=np.int16)        # gather row, 0 pad
        dstv = np.full(tot_e, -1, dtype=np.int16)       # dst-in-block, -1 pad
        eperm = np.full(tot_e, -1, dtype=np.int64)      # orig edge id, -1 pad
        for b in range(NB):
            lo = np.searchsorted(d_c, b * 128, "left")
            hi = np.searchsorted(d_c, (b + 1) * 128, "left")
            n = hi - lo
            o = int(block_off[b])
            ag = (s_c[lo:hi] // NPC) * d["NPCP"] + (s_c[lo:hi] % NPC)
            ag_row[o:o + n] = ag.astype(np.int16)
            dstv[o:o + n] = (d_c[lo:hi] - b * 128).astype(np.int16)
            eperm[o:o + n] = e_c[lo:hi]

        # pack gather indices: per call, idx i -> [i%16, off16 + i//16],
        # replicated across the 8 groups of 16 partitions
        idxp = np.zeros((128, tot_e // 16), dtype=np.int16)
        for (_, off, n) in groups:
            blockv = ag_row[off:off + n].reshape(n // 16, 16).T  # [16, n/16]
            idxp[:16, off // 16: (off + n) // 16] = blockv
        for g in range(1, 8):
            idxp[g * 16:(g + 1) * 16, :] = idxp[:16, :]

        # dstv per chunk column: [128, n_chunks]
        dstv_t = dstv.reshape(tot_e // 128, 128).T.copy()   # [128, NCH]

        prep_cores.append(dict(idxp=idxp, dstv=dstv_t, eperm=eperm))

    structure = dict(cpb=[int(x) for x in cpb], tot_e=tot_e,
                     block_off=[int(x) for x in block_off], groups=groups)
    return structure, prep_cores


# ---------------------------------------------------------------- program

def build_program(cfg, structure, sim_safe=False):
    d = _derived(cfg)
    NC, NB, NPCP, HID, HEADS, DH = (cfg["NCORES"], d["NB"], d["NPCP"],
                                    cfg["HID"], cfg["HEADS"], d["DH"])
    IN_C, L = cfg["IN_C"], cfg["L"]
    cpb, tot_e, block_off = structure["cpb"], structure["tot_e"], structure["block_off"]
    groups = structure["groups"]
    NAG = NC * NPCP
    HSL = HID // 128                                # hid slabs (4)

    nc = bacc.Bacc("TRN2", target_bir_lowering=False, debug=False,
                   num_devices=NC)

    # ---- dram I/O (multi-slab weights shipped already slab-packed on host:
    #      [128, nslab*HID] with slab s = rows [s*128,(s+1)*128) )
    xT0 = nc.dram_tensor("xT0", [128, (IN_C // 128) * NPCP], BF16,
                         kind="ExternalInput")
    Wd = []
    for l in range(L):
        in_c = IN_C if l == 0 else HID
        ins = in_c // 128
        Wd.append({
            **{w: nc.dram_tensor(f"l{l}_{w}", [128, ins * HID], BF16,
                                 kind="ExternalInput")
               for w in ("Wq", "Wk", "Wv", "Wskip")},
            **{w: nc.dram_tensor(f"l{l}_{w}", [128, HSL * HID], BF16,
                                 kind="ExternalInput")
               for w in ("W1", "W2")},
            "We": nc.dram_tensor(f"l{l}_We", [3, HID], BF16,
                                 kind="ExternalInput"),
        })
    idxp_d = nc.dram_tensor("idxp", [128, tot_e // 16], I16, kind="ExternalInput")
    dstv_d = nc.dram_tensor("dstv", [128, tot_e // 128], F32, kind="ExternalInput")
    eaT_d = nc.dram_tensor("eaT", [3, tot_e], BF16, kind="ExternalInput")
    iota_d = nc.dram_tensor("iota", [128, 128], F32, kind="ExternalInput")
    identb_d = nc.dram_tensor("identb", [128, 128], BF16, kind="ExternalInput")
    identf_d = nc.dram_tensor("identf", [128, 128], F32, kind="ExternalInput")
    outT_d = nc.dram_tensor("outT", [HID, NPCP], F32, kind="ExternalOutput")

    # internal dram: per-layer kv shard + allgathered kv table
    kv_shard = [nc.dram_tensor(f"kv_shard{l}", [NPCP, 2 * HID], BF16)
                for l in range(L)]
    kv_full = [nc.dram_tensor(f"kv_full{l}", [NAG, 2 * HID], BF16,
                              addr_space="Shared") for l in range(L)]

    # ---- persistent sbuf
    def sb(name, shape, dt):
        return nc.alloc_sbuf_tensor(name, list(shape), dt)

    xT_sb = [sb("xT0s", [128, (IN_C // 128) * NPCP], BF16),
             sb("xT1s", [128, HSL * NPCP], BF16)]
    w_sb = []
    for l in range(L):
        in_c = IN_C if l == 0 else HID
        ins = in_c // 128
        w_sb.append({
            **{k: sb(f"l{l}{k}s", [128, ins * HID], BF16)
               for k in ("Wq", "Wk", "Wv", "Wskip")},
            **{k: sb(f"l{l}{k}s", [128, HSL * HID], BF16)
               for k in ("W1", "W2")},
            "We": sb(f"l{l}Wes", [3, HID], BF16),
        })
    yT_sb = sb("yTs", [128, HSL * NPCP], BF16)
    idxp_sb = sb("idxps", [128, tot_e // 16], I16)
    dstv_sb = sb("dstvs", [128, tot_e // 128], F32)
    iota_sb = sb("iotas", [128, 128], F32)
    identb_sb = sb("identbs", [128, 128], BF16)
    identf_sb = sb("identfs", [128, 128], F32)

    with tile.TileContext(nc) as tc:
        with (
            tc.tile_pool(name="gath", bufs=4) as p_gath,
            tc.tile_pool(name="chunk", bufs=3) as p_chunk,
            tc.tile_pool(name="small", bufs=4) as p_small,
            tc.tile_pool(name="blk", bufs=2) as p_blk,
            tc.tile_pool(name="ea", bufs=2) as p_ea,
            tc.tile_pool(name="mlp", bufs=3) as p_mlp,
            tc.tile_pool(name="psmm", bufs=2, space="PSUM") as ps_mm,
            tc.tile_pool(name="pschunk", bufs=4, space="PSUM") as ps_chunk,
            tc.tile_pool(name="psnum", bufs=1, space="PSUM") as ps_num,
            tc.tile_pool(name="psden", bufs=1, space="PSUM") as ps_den,
        ):
            def act_gelu(out_ap, in_psum, pool, key):
                """out = gelu_tanh(in). Native ACT table on HW; decomposed
                via Tanh when sim_safe (CoreSim lacks the gelu table)."""
                if not sim_safe:
                    nc.scalar.activation(out_ap, in_psum, AF.Gelu_apprx_tanh)
                    return
                shp = [in_psum.shape[0], in_psum.shape[1]]
                xs = pool.tile(shp, F32, tag="gx", name=f"gx{key}")
                nc.scalar.copy(xs[:], in_psum)
                sq = pool.tile(shp, F32, tag="gs", name=f"gs{key}")
                nc.scalar.square(sq[:], xs[:])
                nc.vector.tensor_scalar(sq[:], sq[:], 0.044715, 1.0,
                                        ALU.mult, ALU.add)
                nc.vector.tensor_tensor(sq[:], sq[:], xs[:], ALU.mult)
                nc.scalar.activation(sq[:], sq[:], AF.Tanh,
                                     scale=0.7978845608028654)
                nc.vector.tensor_scalar(sq[:], sq[:], 1.0, 0.5,
                                        ALU.add, ALU.mult)
                nc.vector.tensor_tensor(out_ap, sq[:], xs[:], ALU.mult)

            # ---- setup loads
            nc.sync.dma_start(xT_sb[0][:, :], xT0[:, :])
            for l in range(L):
                for k in w_sb[l]:
                    nc.sync.dma_start(w_sb[l][k][:, :], Wd[l][k][:, :])
            nc.sync.dma_start(idxp_sb[:, :], idxp_d[:, :])
            nc.sync.dma_start(dstv_sb[:, :], dstv_d[:, :])
            nc.sync.dma_start(iota_sb[:, :], iota_d[:, :])
            nc.sync.dma_start(identb_sb[:, :], identb_d[:, :])
            nc.sync.dma_start(identf_sb[:, :], identf_d[:, :])

            for l in range(L):
                in_c = IN_C if l == 0 else HID
                nslab = in_c // 128
                xT = xT_sb[0] if l == 0 else xT_sb[1]
                W = w_sb[l]

                def dense_block(psum, wname, b, nslab=nslab, xT=xT, W=W):
                    """psum[128,HID] = x[block b] @ W  (node-major out)."""
                    for s in range(nslab):
                        nc.tensor.matmul(
                            psum[:],
                            xT[:, s * NPCP + b * 128: s * NPCP + (b + 1) * 128],
                            W[wname][:, s * HID:(s + 1) * HID],
                            start=(s == 0), stop=(s == nslab - 1))

                # ---------- phase KV
                for b in range(NB):
                    kps = ps_mm.tile([128, HID], F32, tag="mm")
                    dense_block(kps, "Wk", b)
                    vps = ps_mm.tile([128, HID], F32, tag="mm")
                    dense_block(vps, "Wv", b)
                    kvt = p_blk.tile([128, 2 * HID], BF16, tag="kvt")
                    nc.scalar.copy(kvt[:, 0:HID], kps[:])
                    nc.scalar.copy(kvt[:, HID:2 * HID], vps[:])
                    nc.sync.dma_start(kv_shard[l][b * 128:(b + 1) * 128, :], kvt[:])

                nc.gpsimd.collective_compute(
                    "AllGather", ALU.bypass,
                    replica_groups=[list(range(NC))],
                    ins=[kv_shard[l].ap().opt()],
                    outs=[kv_full[l].ap().opt()])

                # ---------- phase ATT
                gtiles = {}   # (block, flat_off) -> tile
                gi = 0
                for b in range(NB):
                    # q / skip for this block
                    qps = ps_mm.tile([128, HID], F32, tag="mm")
                    dense_block(qps, "Wq", b)
                    q_sb = p_blk.tile([128, HID], BF16, tag="qsb")
                    nc.scalar.copy(q_sb[:], qps[:])
                    skp = ps_mm.tile([128, HID], F32, tag="mm")
                    dense_block(skp, "Wskip", b)

                    # edge attrs for the block
                    ne = cpb[b] * 128
                    off = block_off[b]
                    ea_t = p_ea.tile([3, ne], BF16, tag="ea")
                    nc.sync.dma_start(ea_t[:], eaT_d[:, off:off + ne])

                    # gathers for the block
                    while gi < len(groups) and groups[gi][0] == b:
                        _, goff, gn = groups[gi]
                        gt = p_gath.tile([128, gn // 128, 2 * HID], BF16, tag="g")
                        nc.gpsimd.dma_gather(
                            out_ap=gt[:],
                            in_ap=kv_full[l].ap(),
                            idxs_ap=idxp_sb[:, goff // 16:(goff + gn) // 16],
                            num_idxs=gn, num_idxs_reg=gn,
                            elem_size=2 * HID)
                        gtiles[(b, goff)] = gt
                        gi += 1

                    nump = ps_num.tile([128, HID], F32, tag="num")
                    denp = ps_den.tile([128, HEADS], F32, tag="den")

                    for c in range(cpb[b]):
                        ch = (off + c * 128) // 128      # global chunk col
                        goff = off + (c // 4) * 512
                        gt = gtiles[(b, goff)]
                        slot = c % 4
                        kga = gt[:, slot, 0:HID]
                        vga = gt[:, slot, HID:2 * HID]

                        S = p_chunk.tile([128, 128], BF16, tag="S")
                        nc.vector.tensor_scalar(
                            S[:], iota_sb[:, :], dstv_sb[:, ch:ch + 1], None,
                            ALU.is_equal)
                        Gp = ps_chunk.tile([128, 128], BF16, tag="cmm")
                        nc.tensor.transpose(Gp[:], S[:], identb_sb[:, :])
                        G = p_chunk.tile([128, 128], BF16, tag="G")
                        nc.scalar.copy(G[:], Gp[:])

                        eps = ps_chunk.tile([128, HID], F32, tag="cmm")
                        nc.tensor.matmul(eps[:], ea_t[:, c * 128:(c + 1) * 128],
                                         W["We"][:, :], start=True, stop=True)
                        qip = ps_chunk.tile([128, HID], F32, tag="cmm")
                        nc.tensor.matmul(qip[:], G[:], q_sb[:],
                                         start=True, stop=True)

                        kj = p_chunk.tile([128, HID], BF16, tag="kj")
                        nc.vector.tensor_add(kj[:], kga, eps[:])
                        vj = p_chunk.tile([128, HID], BF16, tag="vj")
                        nc.vector.tensor_add(vj[:], vga, eps[:])

                        prod = p_chunk.tile([128, HID], F32, tag="prod")
                        nc.vector.tensor_tensor(prod[:], kj[:], qip[:], ALU.mult)
                        alpha = p_small.tile([128, HEADS], F32, tag="alpha")
                        nc.vector.tensor_reduce(
                            alpha[:],
                            prod[:].rearrange("p (h e) -> p h e", h=HEADS),
                            mybir.AxisListType.X, ALU.add)
                        w_t = p_small.tile([128, HEADS], BF16, tag="w")
                        nc.scalar.activation(w_t[:], alpha[:], AF.Exp,
                                             scale=float(1.0 / np.sqrt(DH)))

                        wv = p_chunk.tile([128, HID], BF16, tag="wv")
                        nc.vector.tensor_tensor(
                            wv[:].rearrange("p (h e) -> p h e", h=HEADS),
                            vj[:].rearrange("p (h e) -> p h e", h=HEADS),
                            w_t[:].unsqueeze(2).broadcast_to([128, HEADS, DH]),
                            ALU.mult)

                        first, last = c == 0, c == cpb[b] - 1
                        nc.tensor.matmul(nump[:], S[:], wv[:],
                                         start=first, stop=last)
                        nc.tensor.matmul(denp[:], S[:], w_t[:],
                                         start=first, stop=last)

                    # softmax denominator + skip
                    den_sb = p_small.tile([128, HEADS], F32, tag="den")
                    nc.vector.tensor_scalar_add(den_sb[:], denp[:], 1e-20)
                    rcp = p_small.tile([128, HEADS], F32, tag="rcp")
                    nc.vector.reciprocal(rcp[:], den_sb[:])
                    y_t = p_blk.tile([128, HID], F32, tag="y")
                    nc.vector.tensor_tensor(
                        y_t[:].rearrange("p (h e) -> p h e", h=HEADS),
                        nump[:].rearrange("p (h e) -> p h e", h=HEADS),
                        rcp[:].unsqueeze(2).broadcast_to([128, HEADS, DH]),
                        ALU.mult)
                    nc.vector.tensor_add(y_t[:], y_t[:], skp[:])

                    # transpose y into yT slabs
                    for j in range(HSL):
                        tp = ps_mm.tile([128, 128], F32, tag="mm")
                        nc.tensor.transpose(tp[:], y_t[:, j * 128:(j + 1) * 128],
                                            identf_sb[:, :])
                        nc.scalar.copy(
                            yT_sb[:, j * NPCP + b * 128: j * NPCP + (b + 1) * 128],
                            tp[:])

                # ---------- phase MLP (transposed layout)
                xTn = xT_sb[1]
                NT = min(512, NPCP)
                for n in range(NPCP // NT):
                    nlo = n * NT
                    h1 = [p_mlp.tile([128, NT], BF16, tag=f"h1_{m}",
                                     name=f"h1_{l}_{n}_{m}")
                          for m in range(HSL)]
                    for m in range(HSL):
                        hp = ps_mm.tile([128, NT], F32, tag="mm")
                        for k in range(HSL):
                            nc.tensor.matmul(
                                hp[:],
                                W["W1"][:, k * HID + m * 128: k * HID + (m + 1) * 128],
                                yT_sb[:, k * NPCP + nlo: k * NPCP + nlo + NT],
                                start=(k == 0), stop=(k == HSL - 1))
                        act_gelu(h1[m][:], hp[:], p_mlp, f"a{l}_{n}_{m}")
                    for m in range(HSL):
                        hp2 = ps_mm.tile([128, NT], F32, tag="mm")
                        for k in range(HSL):
                            nc.tensor.matmul(
                                hp2[:],
                                W["W2"][:, k * HID + m * 128: k * HID + (m + 1) * 128],
                                h1[k][:],
                                start=(k == 0), stop=(k == HSL - 1))
                        g2 = p_mlp.tile([128, NT], F32, tag="g2")
                        act_gelu(g2[:], hp2[:], p_mlp, f"b{l}_{n}_{m}")
                        ot = p_mlp.tile([128, NT], F32, tag="ot")
                        nc.vector.tensor_add(
                            ot[:], g2[:],
                            yT_sb[:, m * NPCP + nlo: m * NPCP + nlo + NT])
                        if l + 1 < L:
                            nc.scalar.copy(
                                xTn[:, m * NPCP + nlo: m * NPCP + nlo + NT],
                                ot[:])
                        else:
                            nc.sync.dma_start(
                                outT_d[m * 128:(m + 1) * 128, nlo:nlo + NT],
                                ot[:])

    nc.compile()
    return nc


# ---------------------------------------------------------------- in-maps

def _slab_pack(w):
    """[in_c, HID] -> [128, (in_c//128)*HID] with slab s at free offset."""
    in_c, hid = w.shape
    ns = in_c // 128
    return np.concatenate([w[s * 128:(s + 1) * 128, :] for s in range(ns)],
                          axis=1)


def make_in_maps(x, edge_attr, params, structure, prep_cores, cfg):
    d = _derived(cfg)
    NC, NPC, NPCP, IN_C = cfg["NCORES"], d["NPC"], d["NPCP"], cfg["IN_C"]
    x = np.asarray(x, np.float32)
    edge_attr = np.asarray(edge_attr, np.float32)

    iota = np.broadcast_to(np.arange(128, dtype=np.float32), (128, 128)).copy()
    ident = np.eye(128, dtype=np.float32)

    shared = {"iota": iota, "identb": ident.astype(NPBF16), "identf": ident}
    for l, p in enumerate(params):
        for k in ("Wq", "Wk", "Wv", "Wskip", "W1", "W2"):
            shared[f"l{l}_{k}"] = _slab_pack(
                np.asarray(p[k], np.float32)).astype(NPBF16)
        shared[f"l{l}_We"] = np.asarray(p["We"], np.float32).astype(NPBF16)
        for k in ("bq", "bk", "bv", "bskip", "b1", "b2"):
            b = np.asarray(p[k], np.float32)
            assert not np.any(b), "nonzero biases not supported by this kernel"

    in_maps = []
    for c in range(NC):
        pc = prep_cores[c]
        xs = np.zeros((IN_C, NPCP), np.float32)
        xs[:, :NPC] = x[c * NPC:(c + 1) * NPC, :].T
        xs = np.concatenate([xs[s * 128:(s + 1) * 128, :]
                             for s in range(IN_C // 128)], axis=1)
        ea = np.zeros((3, structure["tot_e"]), np.float32)
        valid = pc["eperm"] >= 0
        ea[:, valid] = edge_attr[pc["eperm"][valid], :].T
        in_maps.append(shared | {
            "xT0": xs.astype(NPBF16),
            "idxp": pc["idxp"],
            "dstv": pc["dstv"].astype(np.float32),
            "eaT": ea.astype(NPBF16),
        })
    return in_maps


# ---------------------------------------------------------------- entry

_CACHE = {}


def kernel(x, edge_index, edge_attr, params):
    cfg = full_cfg()
    d = _derived(cfg)
    edge_index = np.asarray(edge_index)
    key = hashlib.sha256(edge_index.tobytes()).hexdigest()
    if key not in _CACHE:
        structure, prep_cores = host_prep(edge_index, cfg)
        nc = build_program(cfg, structure)
        _CACHE[key] = (structure, prep_cores, nc)
    structure, prep_cores, nc = _CACHE[key]

    in_maps = make_in_maps(x, edge_attr, params, structure, prep_cores, cfg)
    res = run_bass_kernel_spmd(nc, in_maps, list(range(cfg["NCORES"])))

    out = np.empty((cfg["N"], cfg["HID"]), np.float32)
    for c in range(cfg["NCORES"]):
        out[c * d["NPC"]:(c + 1) * d["NPC"], :] = \
            res.results[c]["outT"][:, :d["NPC"]].T
    return out
